# revision 7
# baseline (speedup 1.0000x reference)
"""Dense matching head (dual-softmax + top-K) on 8 Trainium2 NeuronCores.

Full inputs -> full outputs. Sharding: batch b (4) x row-half h (2) -> core
c = 2*b + h. Each core computes its 2048x4096 block of
P = exp(corr/T) / sqrt(rowsum * colsum); column sums are completed with a
16 KB pair AllReduce. Matmuls run as hi/lo-split float32r (full fp32
accuracy at 1 cycle/row). Row-max statistics come back per core; the final
top-K selection and argmax-gather run on host over O(B*N + K*N) data.
"""

import sys

sys.path.insert(0, "/opt/trn_rl_repo")

import numpy as np

B, H, W, C = 4, 64, 64, 128
N = H * W  # 4096
RHALF = N // 2  # 2048
NT = RHALF // 128  # 16 row tiles per core
TEMPERATURE = 0.1
K = 100
N_CORES = 8

_cached = {}


def _build_module():
    import concourse.bacc as bacc
    import concourse.mybir as mybir
    import concourse.tile as tile
    from concourse.masks import make_identity

    f32 = mybir.dt.float32
    f32r = mybir.dt.float32r
    AF = mybir.ActivationFunctionType
    AL = mybir.AluOpType

    nc = bacc.Bacc(
        "TRN2", target_bir_lowering=False, debug=False, num_devices=N_CORES
    )

    f1_ap = nc.dram_tensor("f1h", [RHALF, C], f32, kind="ExternalInput").ap()
    f2_ap = nc.dram_tensor("f2f", [N, C], f32, kind="ExternalInput").ap()
    p_ap = nc.dram_tensor("p_out", [RHALF, N], f32, kind="ExternalOutput").ap()
    rm_ap = nc.dram_tensor("rm_out", [128, NT], f32, kind="ExternalOutput").ap()
    rb_ap = nc.dram_tensor("rb_out", [128, NT], f32, kind="ExternalOutput").ap()

    NT1 = NT  # f1 tiles
    NT2 = N // 128  # 32 f2 tiles

    with tile.TileContext(nc) as tc, tc.tile_pool(name="persist", bufs=1) as pp:
        # ---------------- persistent tiles ----------------
        f1hi = pp.tile([128, RHALF], f32r)
        f1lo = pp.tile([128, RHALF], f32r)
        f2hi = pp.tile([128, N], f32r)
        f2lo = pp.tile([128, N], f32r)
        cs_acc = pp.tile([128, N], f32)
        rs_parts = pp.tile([128, 2 * NT], f32)
        rm_parts = pp.tile([128, 2 * NT], f32)
        rs_all = pp.tile([128, NT], f32)
        rb_all = pp.tile([128, NT], f32)
        rm_all = pp.tile([128, NT], f32)
        ones128r = pp.tile([128, 128], f32r)
        ones2r = pp.tile([2, 128], f32r)
        nlncs2 = pp.tile([2, N], f32r)

        # ---------------- stage A: load, normalize, transpose ----------------
        with (
            tc.tile_pool(name="sbA", bufs=3) as pa,
            tc.tile_pool(name="sbT", bufs=1) as pt,
            tc.tile_pool(name="psA", bufs=2, space="PSUM") as psA,
        ):
            f1T = pt.tile([128, RHALF], f32)
            f2T = pt.tile([128, N], f32)
            ident = pp.tile([128, 128], f32)
            make_identity(nc, ident[:])
            onesf = pp.tile([128, 128], f32)
            nc.gpsimd.memset(onesf[:], 1.0)
            nc.scalar.copy(ones128r[:], onesf[:])
            nc.scalar.copy(ones2r[:], onesf[0:2, :])

            q1 = pp.tile([128, NT1], f32)
            q2 = pp.tile([128, NT2], f32)

            def load_tile(i, tag):
                t = pa.tile([128, C], f32, tag=tag)
                if i < NT1:
                    nc.sync.dma_start(t[:], f1_ap[i * 128 : (i + 1) * 128, :])
                else:
                    j = i - NT1
                    nc.sync.dma_start(t[:], f2_ap[j * 128 : (j + 1) * 128, :])
                return t

            for i in range(NT1 + NT2):
                t = load_tile(i, "inq")
                sc = pa.tile([128, C], f32, tag="sc")
                q = q1[:, i : i + 1] if i < NT1 else q2[:, i - NT1 : i - NT1 + 1]
                nc.scalar.activation(sc[:], t[:], AF.Square, accum_out=q)

            # rn = 1/sqrt(q) with one Heron refinement of sqrt
            def rsqrt_chain(q, n):
                s0 = pp.tile([128, n], f32)
                nc.scalar.sqrt(s0[:], q[:])
                r0 = pp.tile([128, n], f32)
                nc.vector.reciprocal(r0[:], s0[:])
                t1 = pp.tile([128, n], f32)
                nc.vector.tensor_mul(t1[:], q[:], r0[:])
                nc.vector.tensor_add(t1[:], t1[:], s0[:])
                nc.vector.tensor_scalar_mul(t1[:], t1[:], 0.5)
                rn = pp.tile([128, n], f32)
                nc.vector.reciprocal(rn[:], t1[:])
                return rn

            rn1 = rsqrt_chain(q1, NT1)
            rn2 = rsqrt_chain(q2, NT2)

            # scale rows (f1 also x 1/TEMPERATURE), transpose, pack
            for i in range(NT1 + NT2):
                t = load_tile(i, "ins")
                if i < NT1:
                    rn = rn1[:, i : i + 1]
                    scl2 = 1.0 / TEMPERATURE
                else:
                    rn = rn2[:, i - NT1 : i - NT1 + 1]
                    scl2 = 1.0
                sc = pa.tile([128, C], f32, tag="nm")
                nc.vector.tensor_scalar(
                    out=sc[:], in0=t[:], scalar1=rn, scalar2=scl2,
                    op0=AL.mult, op1=AL.mult,
                )
                if i % 4 == 0:
                    tp = psA.tile([128, 512], f32, tag="tp")
                nc.tensor.transpose(tp[:, (i % 4) * 128 : (i % 4 + 1) * 128], sc[:], ident[:])
                if i % 4 == 3 or i == NT1 + NT2 - 1:
                    # flush the psum quad with one DVE copy
                    nquad = (i % 4) + 1
                    base = (i - (i % 4))
                    if base < NT1:
                        nc.vector.tensor_copy(
                            f1T[:, base * 128 : base * 128 + nquad * 128],
                            tp[:, : nquad * 128],
                        )
                    else:
                        b2 = base - NT1
                        nc.vector.tensor_copy(
                            f2T[:, b2 * 128 : b2 * 128 + nquad * 128],
                            tp[:, : nquad * 128],
                        )

            # hi/lo split (full width ops)
            hb_scratch = pt.tile([128, N], f32)

            def hilo(srcT, hi, lo, width):
                nc.vector.tensor_copy(hi[:], srcT[:])  # f32 -> f32r round
                hi_back = hb_scratch[:, :width]
                nc.vector.tensor_copy(hi_back, hi[:])  # decode
                nc.vector.tensor_tensor(
                    out=srcT[:], in0=srcT[:], in1=hi_back, op=AL.subtract
                )
                nc.vector.tensor_copy(lo[:], srcT[:])

            hilo(f1T, f1hi, f1lo, RHALF)
            hilo(f2T, f2hi, f2lo, N)

        # ---------------- stage B: pass 1 (exp sums) ----------------
        with (
            tc.tile_pool(name="sbB", bufs=3) as pb,
            tc.tile_pool(name="psB", bufs=2, space="PSUM") as psB,
        ):
            for t in range(NT):
                fc = t * 128
                for half in range(2):
                    hp = psB.tile([128, 2048], f32, tag="hp")
                    for c4 in range(4):
                        off = half * 2048 + c4 * 512
                        o = hp[:, c4 * 512 : (c4 + 1) * 512]
                        nc.tensor.matmul(
                            o, f1hi[:, fc : fc + 128], f2hi[:, off : off + 512],
                            start=True, stop=False,
                        )
                        nc.tensor.matmul(
                            o, f1hi[:, fc : fc + 128], f2lo[:, off : off + 512],
                            start=False, stop=False,
                        )
                        nc.tensor.matmul(
                            o, f1lo[:, fc : fc + 128], f2hi[:, off : off + 512],
                            start=False, stop=True,
                        )
                    e = pb.tile([128, 2048], f32, tag="e")
                    nc.scalar.activation(
                        e[:], hp[:], AF.Exp,
                        accum_out=rs_parts[:, 2 * t + half : 2 * t + half + 1],
                    )
                    dst = cs_acc[:, half * 2048 : (half + 1) * 2048]
                    if t == 0:
                        nc.vector.tensor_copy(dst, e[:])
                    else:
                        nc.vector.tensor_add(dst, dst, e[:])

        # ---------------- stage C: stats + pair allreduce ----------------
        with (
            tc.tile_pool(name="sbC", bufs=1) as pc,
            tc.tile_pool(name="psC", bufs=1, space="PSUM") as psC,
            tc.tile_pool(name="dramC", bufs=1, space="DRAM") as dc,
        ):
            # rs_all = even + odd slots ; rb = -0.5*ln(rs)
            rsp3 = rs_parts[:].rearrange("p (t two) -> p t two", two=2)
            nc.vector.tensor_tensor(
                out=rs_all[:], in0=rsp3[:, :, 0], in1=rsp3[:, :, 1], op=AL.add
            )
            nc.scalar.activation(rb_all[:], rs_all[:], AF.Ln)
            nc.vector.tensor_scalar_mul(rb_all[:], rb_all[:], -0.5)
            nc.sync.dma_start(rb_ap[:], rb_all[:])

            # cs partial: partition-reduce via ones-matmul
            csr = pc.tile([128, N], f32r)
            nc.vector.tensor_copy(csr[:], cs_acc[:])
            csback = pc.tile([128, N], f32, tag="big2")
            nc.vector.tensor_copy(csback[:], csr[:])
            nc.vector.tensor_tensor(
                out=cs_acc[:], in0=cs_acc[:], in1=csback[:], op=AL.subtract
            )
            csr_lo = pc.tile([128, N], f32r, tag="big2")
            nc.vector.tensor_copy(csr_lo[:], cs_acc[:])
            csps = psC.tile([128, N], f32)
            for c8 in range(8):
                sl = slice(c8 * 512, (c8 + 1) * 512)
                nc.tensor.matmul(
                    csps[:, sl], ones128r[:], csr[:, sl], start=True, stop=False
                )
                nc.tensor.matmul(
                    csps[:, sl], ones128r[:], csr_lo[:, sl], start=False, stop=True
                )
            cs_part = pc.tile([1, N], f32, tag="row1")
            nc.scalar.copy(cs_part[:], csps[0:1, :])

            crin = dc.tile([1, N], f32)
            crout = dc.tile([1, N], f32)
            nc.sync.dma_start(crin[:], cs_part[:])
            nc.gpsimd.collective_compute(
                "AllReduce",
                AL.add,
                replica_groups=[[0, 1], [2, 3], [4, 5], [6, 7]],
                ins=[crin.opt()],
                outs=[crout.opt()],
            )
            cs_row = pc.tile([1, N], f32, tag="row2")
            nc.sync.dma_start(cs_row[:], crout[:])

            nlncs_f = pc.tile([1, N], f32, tag="row3")
            nc.scalar.activation(nlncs_f[:], cs_row[:], AF.Ln)
            nc.vector.tensor_scalar_mul(nlncs_f[:], nlncs_f[:], -0.5)
            nc.vector.tensor_copy(nlncs2[0:1, :], nlncs_f[:])
            nback = pc.tile([1, N], f32, tag="row1")
            nc.vector.tensor_copy(nback[:], nlncs2[0:1, :])
            nc.vector.tensor_tensor(
                out=nlncs_f[:], in0=nlncs_f[:], in1=nback[:], op=AL.subtract
            )
            nlo_r = pc.tile([1, N], f32r, tag="row2")
            nc.vector.tensor_copy(nlo_r[:], nlncs_f[:])
            nc.sync.dma_start(nlncs2[1:2, :], nlo_r[:])

        # ---------------- stage D: pass 2 (P + row max) ----------------
        with (
            tc.tile_pool(name="sbD", bufs=3) as pd,
            tc.tile_pool(name="psD", bufs=2, space="PSUM") as psD,
        ):
            for t in range(NT):
                fc = t * 128
                for half in range(2):
                    hp = psD.tile([128, 2048], f32, tag="hp2")
                    for c4 in range(4):
                        off = half * 2048 + c4 * 512
                        o = hp[:, c4 * 512 : (c4 + 1) * 512]
                        nc.tensor.matmul(
                            o, f1hi[:, fc : fc + 128], f2hi[:, off : off + 512],
                            start=True, stop=False,
                        )
                        nc.tensor.matmul(
                            o, f1hi[:, fc : fc + 128], f2lo[:, off : off + 512],
                            start=False, stop=False,
                        )
                        nc.tensor.matmul(
                            o, f1lo[:, fc : fc + 128], f2hi[:, off : off + 512],
                            start=False, stop=False,
                        )
                        nc.tensor.matmul(
                            o, ones2r[:], nlncs2[:, off : off + 512],
                            start=False, stop=True,
                        )
                    import concourse.mybir as _mb

                    nc.vector.reduce_max(
                        rm_parts[:, 2 * t + half : 2 * t + half + 1], hp[:],
                        axis=_mb.AxisListType.X,
                    )
                    p_sb = pd.tile([128, 2048], f32, tag="p")
                    nc.scalar.activation(
                        p_sb[:], hp[:], AF.Exp, bias=rb_all[:, t : t + 1]
                    )
                    nc.sync.dma_start(
                        p_ap[t * 128 : (t + 1) * 128, half * 2048 : (half + 1) * 2048],
                        p_sb[:],
                    )
            rmp3 = rm_parts[:].rearrange("p (t two) -> p t two", two=2)
            nc.vector.tensor_tensor(
                out=rm_all[:], in0=rmp3[:, :, 0], in1=rmp3[:, :, 1], op=AL.max
            )
            nc.sync.dma_start(rm_ap[:], rm_all[:])

    nc.compile()
    return nc


def get_module():
    if "nc" not in _cached:
        _cached["nc"] = _build_module()
    return _cached["nc"]


def make_in_maps(feat1, feat2):
    f1 = np.ascontiguousarray(np.asarray(feat1, dtype=np.float32)).reshape(B, N, C)
    f2 = np.ascontiguousarray(np.asarray(feat2, dtype=np.float32)).reshape(B, N, C)
    in_maps = []
    for c in range(N_CORES):
        b, half = c // 2, c % 2
        in_maps.append(
            {
                "f1h": np.ascontiguousarray(f1[b, half * RHALF : (half + 1) * RHALF]),
                "f2f": f2[b],
            }
        )
    return in_maps


def postprocess(results):
    """results: list of 8 dicts with p_out/rm_out/rb_out -> (P, matches)."""
    P = np.empty((B, N, N), dtype=np.float32)
    matches = np.empty((B, K, 2), dtype=np.int32)
    for b in range(B):
        score_halves = []
        for half in range(2):
            r = results[2 * b + half]
            P[b, half * RHALF : (half + 1) * RHALF] = r["p_out"]
            # [128, NT] with row index = t*128 + p  ->  flatten as t-major
            s = (
                r["rm_out"].astype(np.float64) + r["rb_out"].astype(np.float64)
            ).T.reshape(RHALF)
            score_halves.append(s)
        score = np.concatenate(score_halves)
        # mimic reference: stable argsort over f32 max-values, descending
        mv32 = np.exp(score).astype(np.float32)
        top = np.argsort(-mv32, kind="stable")[:K]
        mj = np.argmax(P[b][top], axis=1)
        matches[b, :, 0] = top.astype(np.int32)
        matches[b, :, 1] = mj.astype(np.int32)
    return P, matches


def kernel(feat1, feat2):
    from concourse.bass_utils import run_bass_kernel_spmd

    nc = get_module()
    in_maps = make_in_maps(feat1, feat2)
    res = run_bass_kernel_spmd(nc, in_maps, core_ids=list(range(N_CORES)))
    return postprocess(res.results)


# revision 9
# speedup vs baseline: 189.0017x; 189.0017x over previous
"""Dense matching head (dual-softmax + top-K) on 8 Trainium2 NeuronCores.

Full inputs -> full outputs. Sharding: batch b (4) x row-half h (2) -> core
c = 2*b + h. Each core computes its 2048x4096 block of
P = exp(corr/T) / sqrt(rowsum * colsum); column sums are completed with a
16 KB pair AllReduce. Matmuls run as hi/lo-split float32r (full fp32
accuracy at 1 cycle/row). Row-max statistics come back per core; the final
top-K selection and argmax-gather run on host over O(B*N + K*N) data.
"""

import sys

sys.path.insert(0, "/opt/trn_rl_repo")

import numpy as np

B, H, W, C = 4, 64, 64, 128
N = H * W  # 4096
RHALF = N // 2  # 2048
NT = RHALF // 128  # 16 row tiles per core
TEMPERATURE = 0.1
K = 100
N_CORES = 8

_cached = {}


def _build_module(with_collective=True, num_devices=N_CORES):
    import concourse.bacc as bacc
    import concourse.mybir as mybir
    import concourse.tile as tile
    from concourse.masks import make_identity

    f32 = mybir.dt.float32
    f32r = mybir.dt.float32r
    AF = mybir.ActivationFunctionType
    AL = mybir.AluOpType

    nc = bacc.Bacc(
        "TRN2", target_bir_lowering=False, debug=False, num_devices=num_devices
    )

    f1_ap = nc.dram_tensor("f1h", [RHALF, C], f32, kind="ExternalInput").ap()
    f2_ap = nc.dram_tensor("f2f", [N, C], f32, kind="ExternalInput").ap()
    p_ap = nc.dram_tensor("p_out", [RHALF, N], f32, kind="ExternalOutput").ap()
    rm_ap = nc.dram_tensor("rm_out", [128, NT], f32, kind="ExternalOutput").ap()
    rb_ap = nc.dram_tensor("rb_out", [128, NT], f32, kind="ExternalOutput").ap()

    NT1 = NT  # f1 tiles
    NT2 = N // 128  # 32 f2 tiles

    with tile.TileContext(nc) as tc, tc.tile_pool(name="persist", bufs=1) as pp:
        # ---------------- persistent tiles ----------------
        f1hi = pp.tile([128, RHALF], f32r)
        f1lo = pp.tile([128, RHALF], f32r)
        f2hi = pp.tile([128, N], f32r)
        f2lo = pp.tile([128, N], f32r)
        cs_acc = pp.tile([128, N], f32)
        rs_parts = pp.tile([128, 2 * NT], f32)
        rm_parts = pp.tile([128, 2 * NT], f32)
        rs_all = pp.tile([128, NT], f32)
        rb_all = pp.tile([128, NT], f32)
        rm_all = pp.tile([128, NT], f32)
        ones128r = pp.tile([128, 128], f32r)
        ones2r = pp.tile([2, 128], f32r)
        nlncs2 = pp.tile([2, N], f32r)

        # ---------------- stage A: load, normalize, transpose ----------------
        with (
            tc.tile_pool(name="sbA", bufs=3) as pa,
            tc.tile_pool(name="sbT", bufs=1) as pt,
            tc.tile_pool(name="psA", bufs=2, space="PSUM") as psA,
        ):
            f1T = pt.tile([128, RHALF], f32)
            f2T = pt.tile([128, N], f32)
            ident = pp.tile([128, 128], f32)
            make_identity(nc, ident[:])
            onesf = pp.tile([128, 128], f32)
            nc.gpsimd.memset(onesf[:], 1.0)
            nc.scalar.copy(ones128r[:], onesf[:])
            nc.scalar.copy(ones2r[:], onesf[0:2, :])

            q1 = pp.tile([128, NT1], f32)
            q2 = pp.tile([128, NT2], f32)

            def load_tile(i, tag):
                t = pa.tile([128, C], f32, tag=tag)
                if i < NT1:
                    nc.sync.dma_start(t[:], f1_ap[i * 128 : (i + 1) * 128, :])
                else:
                    j = i - NT1
                    nc.sync.dma_start(t[:], f2_ap[j * 128 : (j + 1) * 128, :])
                return t

            for i in range(NT1 + NT2):
                t = load_tile(i, "inq")
                sc = pa.tile([128, C], f32, tag="sc")
                q = q1[:, i : i + 1] if i < NT1 else q2[:, i - NT1 : i - NT1 + 1]
                nc.scalar.activation(sc[:], t[:], AF.Square, accum_out=q)

            # rn = 1/sqrt(q) with one Heron refinement of sqrt
            def rsqrt_chain(q, n):
                s0 = pp.tile([128, n], f32)
                nc.scalar.sqrt(s0[:], q[:])
                r0 = pp.tile([128, n], f32)
                nc.vector.reciprocal(r0[:], s0[:])
                t1 = pp.tile([128, n], f32)
                nc.vector.tensor_mul(t1[:], q[:], r0[:])
                nc.vector.tensor_add(t1[:], t1[:], s0[:])
                nc.vector.tensor_scalar_mul(t1[:], t1[:], 0.5)
                rn = pp.tile([128, n], f32)
                nc.vector.reciprocal(rn[:], t1[:])
                return rn

            rn1 = rsqrt_chain(q1, NT1)
            rn2 = rsqrt_chain(q2, NT2)

            # scale rows (f1 also x 1/TEMPERATURE), transpose, pack
            for i in range(NT1 + NT2):
                t = load_tile(i, "ins")
                if i < NT1:
                    rn = rn1[:, i : i + 1]
                    scl2 = 1.0 / TEMPERATURE
                else:
                    rn = rn2[:, i - NT1 : i - NT1 + 1]
                    scl2 = 1.0
                sc = pa.tile([128, C], f32, tag="nm")
                nc.vector.tensor_scalar(
                    out=sc[:], in0=t[:], scalar1=rn, scalar2=scl2,
                    op0=AL.mult, op1=AL.mult,
                )
                if i % 4 == 0:
                    tp = psA.tile([128, 512], f32, tag="tp")
                nc.tensor.transpose(tp[:, (i % 4) * 128 : (i % 4 + 1) * 128], sc[:], ident[:])
                if i % 4 == 3 or i == NT1 + NT2 - 1:
                    # flush the psum quad with one DVE copy
                    nquad = (i % 4) + 1
                    base = (i - (i % 4))
                    if base < NT1:
                        nc.vector.tensor_copy(
                            f1T[:, base * 128 : base * 128 + nquad * 128],
                            tp[:, : nquad * 128],
                        )
                    else:
                        b2 = base - NT1
                        nc.vector.tensor_copy(
                            f2T[:, b2 * 128 : b2 * 128 + nquad * 128],
                            tp[:, : nquad * 128],
                        )

            # hi/lo split (full width ops)
            hb_scratch = pt.tile([128, N], f32)

            def hilo(srcT, hi, lo, width):
                nc.vector.tensor_copy(hi[:], srcT[:])  # f32 -> f32r round
                hi_back = hb_scratch[:, :width]
                nc.vector.tensor_copy(hi_back, hi[:])  # decode
                nc.vector.tensor_tensor(
                    out=srcT[:], in0=srcT[:], in1=hi_back, op=AL.subtract
                )
                nc.vector.tensor_copy(lo[:], srcT[:])

            hilo(f1T, f1hi, f1lo, RHALF)
            hilo(f2T, f2hi, f2lo, N)

        # ---------------- stage B: pass 1 (exp sums) ----------------
        with (
            tc.tile_pool(name="sbB", bufs=3) as pb,
            tc.tile_pool(name="psB", bufs=2, space="PSUM") as psB,
        ):
            for t in range(NT):
                fc = t * 128
                for half in range(2):
                    hp = psB.tile([128, 2048], f32, tag="hp")
                    for c4 in range(4):
                        off = half * 2048 + c4 * 512
                        o = hp[:, c4 * 512 : (c4 + 1) * 512]
                        nc.tensor.matmul(
                            o, f1hi[:, fc : fc + 128], f2hi[:, off : off + 512],
                            start=True, stop=False,
                        )
                        nc.tensor.matmul(
                            o, f1hi[:, fc : fc + 128], f2lo[:, off : off + 512],
                            start=False, stop=False,
                        )
                        nc.tensor.matmul(
                            o, f1lo[:, fc : fc + 128], f2hi[:, off : off + 512],
                            start=False, stop=True,
                        )
                    e = pb.tile([128, 2048], f32, tag="e")
                    nc.scalar.activation(
                        e[:], hp[:], AF.Exp,
                        accum_out=rs_parts[:, 2 * t + half : 2 * t + half + 1],
                    )
                    dst = cs_acc[:, half * 2048 : (half + 1) * 2048]
                    if t == 0:
                        nc.vector.tensor_copy(dst, e[:])
                    else:
                        nc.vector.tensor_add(dst, dst, e[:])

        # ---------------- stage C: stats + pair allreduce ----------------
        with (
            tc.tile_pool(name="sbC", bufs=1) as pc,
            tc.tile_pool(name="psC", bufs=1, space="PSUM") as psC,
            tc.tile_pool(name="dramC", bufs=1, space="DRAM") as dc,
        ):
            # rs_all = even + odd slots ; rb = -0.5*ln(rs)
            rsp3 = rs_parts[:].rearrange("p (t two) -> p t two", two=2)
            nc.vector.tensor_tensor(
                out=rs_all[:], in0=rsp3[:, :, 0], in1=rsp3[:, :, 1], op=AL.add
            )
            nc.scalar.activation(rb_all[:], rs_all[:], AF.Ln)
            nc.vector.tensor_scalar_mul(rb_all[:], rb_all[:], -0.5)
            nc.sync.dma_start(rb_ap[:], rb_all[:])

            # cs partial: partition-reduce via ones-matmul
            csr = pc.tile([128, N], f32r)
            nc.vector.tensor_copy(csr[:], cs_acc[:])
            csback = pc.tile([128, N], f32, tag="big2")
            nc.vector.tensor_copy(csback[:], csr[:])
            nc.vector.tensor_tensor(
                out=cs_acc[:], in0=cs_acc[:], in1=csback[:], op=AL.subtract
            )
            csr_lo = pc.tile([128, N], f32r, tag="big2")
            nc.vector.tensor_copy(csr_lo[:], cs_acc[:])
            csps = psC.tile([128, N], f32)
            for c8 in range(8):
                sl = slice(c8 * 512, (c8 + 1) * 512)
                nc.tensor.matmul(
                    csps[:, sl], ones128r[:], csr[:, sl], start=True, stop=False
                )
                nc.tensor.matmul(
                    csps[:, sl], ones128r[:], csr_lo[:, sl], start=False, stop=True
                )
            cs_part = pc.tile([1, N], f32, tag="row1")
            nc.scalar.copy(cs_part[:], csps[0:1, :])

            crin = dc.tile([1, N], f32)
            crout = dc.tile([1, N], f32)
            nc.sync.dma_start(crin[:], cs_part[:])
            if with_collective:
                nc.gpsimd.collective_compute(
                    "AllReduce",
                    AL.add,
                    replica_groups=[[0, 1], [2, 3], [4, 5], [6, 7]],
                    ins=[crin.opt()],
                    outs=[crout.opt()],
                )
            else:
                nc.sync.dma_start(crout[:], crin[:])
            cs_row = pc.tile([1, N], f32, tag="row2")
            nc.sync.dma_start(cs_row[:], crout[:])

            nlncs_f = pc.tile([1, N], f32, tag="row3")
            nc.scalar.activation(nlncs_f[:], cs_row[:], AF.Ln)
            nc.vector.tensor_scalar_mul(nlncs_f[:], nlncs_f[:], -0.5)
            nc.vector.tensor_copy(nlncs2[0:1, :], nlncs_f[:])
            nback = pc.tile([1, N], f32, tag="row1")
            nc.vector.tensor_copy(nback[:], nlncs2[0:1, :])
            nc.vector.tensor_tensor(
                out=nlncs_f[:], in0=nlncs_f[:], in1=nback[:], op=AL.subtract
            )
            nlo_r = pc.tile([1, N], f32r, tag="row2")
            nc.vector.tensor_copy(nlo_r[:], nlncs_f[:])
            nc.sync.dma_start(nlncs2[1:2, :], nlo_r[:])

        # ---------------- stage D: pass 2 (P + row max) ----------------
        with (
            tc.tile_pool(name="sbD", bufs=3) as pd,
            tc.tile_pool(name="psD", bufs=2, space="PSUM") as psD,
        ):
            for t in range(NT):
                fc = t * 128
                for half in range(2):
                    hp = psD.tile([128, 2048], f32, tag="hp2")
                    for c4 in range(4):
                        off = half * 2048 + c4 * 512
                        o = hp[:, c4 * 512 : (c4 + 1) * 512]
                        nc.tensor.matmul(
                            o, f1hi[:, fc : fc + 128], f2hi[:, off : off + 512],
                            start=True, stop=False,
                        )
                        nc.tensor.matmul(
                            o, f1hi[:, fc : fc + 128], f2lo[:, off : off + 512],
                            start=False, stop=False,
                        )
                        nc.tensor.matmul(
                            o, f1lo[:, fc : fc + 128], f2hi[:, off : off + 512],
                            start=False, stop=False,
                        )
                        nc.tensor.matmul(
                            o, ones2r[:], nlncs2[:, off : off + 512],
                            start=False, stop=True,
                        )
                    import concourse.mybir as _mb

                    nc.vector.reduce_max(
                        rm_parts[:, 2 * t + half : 2 * t + half + 1], hp[:],
                        axis=_mb.AxisListType.X,
                    )
                    p_sb = pd.tile([128, 2048], f32, tag="p")
                    nc.scalar.activation(
                        p_sb[:], hp[:], AF.Exp, bias=rb_all[:, t : t + 1]
                    )
                    nc.sync.dma_start(
                        p_ap[t * 128 : (t + 1) * 128, half * 2048 : (half + 1) * 2048],
                        p_sb[:],
                    )
            rmp3 = rm_parts[:].rearrange("p (t two) -> p t two", two=2)
            nc.vector.tensor_tensor(
                out=rm_all[:], in0=rmp3[:, :, 0], in1=rmp3[:, :, 1], op=AL.max
            )
            nc.sync.dma_start(rm_ap[:], rm_all[:])

    nc.compile()
    return nc


def get_module():
    if "nc" not in _cached:
        _cached["nc"] = _build_module()
    return _cached["nc"]


def make_in_maps(feat1, feat2):
    f1 = np.ascontiguousarray(np.asarray(feat1, dtype=np.float32)).reshape(B, N, C)
    f2 = np.ascontiguousarray(np.asarray(feat2, dtype=np.float32)).reshape(B, N, C)
    in_maps = []
    for c in range(N_CORES):
        b, half = c // 2, c % 2
        in_maps.append(
            {
                "f1h": np.ascontiguousarray(f1[b, half * RHALF : (half + 1) * RHALF]),
                "f2f": f2[b],
            }
        )
    return in_maps


def postprocess(results):
    """results: list of 8 dicts with p_out/rm_out/rb_out -> (P, matches)."""
    P = np.empty((B, N, N), dtype=np.float32)
    matches = np.empty((B, K, 2), dtype=np.int32)
    for b in range(B):
        score_halves = []
        for half in range(2):
            r = results[2 * b + half]
            P[b, half * RHALF : (half + 1) * RHALF] = r["p_out"]
            # [128, NT] with row index = t*128 + p  ->  flatten as t-major
            s = (
                r["rm_out"].astype(np.float64) + r["rb_out"].astype(np.float64)
            ).T.reshape(RHALF)
            score_halves.append(s)
        score = np.concatenate(score_halves)
        # mimic reference: stable argsort over f32 max-values, descending
        mv32 = np.exp(score).astype(np.float32)
        top = np.argsort(-mv32, kind="stable")[:K]
        mj = np.argmax(P[b][top], axis=1)
        matches[b, :, 0] = top.astype(np.int32)
        matches[b, :, 1] = mj.astype(np.int32)
    return P, matches


def kernel(feat1, feat2):
    from concourse.bass_utils import run_bass_kernel_spmd

    nc = get_module()
    in_maps = make_in_maps(feat1, feat2)
    res = run_bass_kernel_spmd(nc, in_maps, core_ids=list(range(N_CORES)))
    return postprocess(res.results)
